# revision 1
# baseline (speedup 1.0000x reference)
"""Trainium2 Bass kernel for a cross-attention block (B=2, C=128, H=W=64, 4 heads).

Sharding: one (batch, head) pair per NeuronCore (2*4 = 8 cores).  Each core:
  - group-norms x[b] / context[b] (stats only; the affine normalization is
    folded into the projection weights),
  - computes its head's q, k, v projections,
  - runs softmax(q^T k / sqrt(hd)) @ v^T with the score matrix streamed
    through PSUM (never materialized in HBM),
  - applies its head's slice of the output projection.
The host sums the 4 per-head partial outputs of each batch (the residual x
and bias are added on exactly one core per batch via the `resw` input, so the
sum is a pure unshard).

Layout notes:
  - Scores are computed transposed (e on partitions, d free) so softmax
    normalization uses a ones-row appended to v^T (column sums fall out of
    the same matmul as attn@v) and no transposes are needed anywhere.
  - S^T matmuls have contraction dim 32 (head dim); the 4 heads... rather,
    4 consecutive e-tiles are packed into the 4 PE row groups
    (tile_position) so they run concurrently: k is produced in a
    "distributed" layout (e-tile eo lives on partitions 32*(eo%4)..) and q
    replicated on all 4 partition groups, both directly from the projection
    matmuls at no extra cost (wq4 = 4x tiled wqT, wk4 = per-group masked).
  - Matmul inputs are bitcast to float32r (1 cycle/row vs 4 for fp32).
"""

import os
import numpy as np

import concourse.bass as bass
import concourse.bacc as bacc
import concourse.tile as tile
import concourse.mybir as mybir
from concourse.bass import ts
from concourse.bass_utils import run_bass_kernel_spmd

F32 = mybir.dt.float32
F32R = mybir.dt.float32r
BF16 = mybir.dt.bfloat16
AF = mybir.ActivationFunctionType
OP = mybir.AluOpType

B, C, H, W = 2, 128, 64, 64
HW = H * W            # 4096
NH = 4                # heads
HD = C // NH          # 32
NG = 32               # groupnorm groups
EPS = 1e-5
NE = HW // 128        # 32 e-tiles of 128
D = 512               # d-chunk (query positions per chunk)
ND = HW // D          # 8 chunks
SCALE = float(1.0 / np.sqrt(HD))
# exp groups per chunk: e-tiles per S-psum fill; each e-tile's (128, 512)
# score block fills exactly one PSUM bank (concurrent row-group matmuls
# must hit distinct, bank-aligned banks).  spA = 4 banks, spB = 3 banks.
EXP_GROUPS = [(4, "A"), (2, "B"), (4, "A"), (2, "B"), (4, "A"), (2, "B"),
              (4, "A"), (2, "B"), (4, "A"), (2, "B"), (2, "B")]


def _r(ap):
    return ap.bitcast(F32R)


def _build_module():
    nc = bacc.Bacc("TRN2", target_bir_lowering=False)

    x_d = nc.dram_tensor("x", (C, HW), F32R, kind="ExternalInput")
    ctx_d = nc.dram_tensor("ctx", (C, HW), F32R, kind="ExternalInput")
    wq4_d = nc.dram_tensor("wq4", (C, C), F32R, kind="ExternalInput")
    wk4_d = nc.dram_tensor("wk4", (C, NH, C), F32R, kind="ExternalInput")
    wvt_d = nc.dram_tensor("wvt", (C, HD), F32R, kind="ExternalInput")
    wot_d = nc.dram_tensor("wot", (HD, C), F32R, kind="ExternalInput")
    gsel_d = nc.dram_tensor("gsel", (C, C), F32, kind="ExternalInput")
    gq_d = nc.dram_tensor("gq", (C, 1), F32, kind="ExternalInput")
    bq_d = nc.dram_tensor("bq", (C, 1), F32, kind="ExternalInput")
    gc_d = nc.dram_tensor("gc", (C, 1), F32, kind="ExternalInput")
    bc_d = nc.dram_tensor("bc", (C, 1), F32, kind="ExternalInput")
    bo_d = nc.dram_tensor("bo", (C, 1), F32, kind="ExternalInput")
    al_d = nc.dram_tensor("al", (1, 1), F32, kind="ExternalInput")
    rw_d = nc.dram_tensor("rw", (1, 1), F32, kind="ExternalInput")
    y_d = nc.dram_tensor("y", (C, HW), F32, kind="ExternalOutput")

    with tile.TileContext(nc) as tc:
        with (
            tc.tile_pool(name="const", bufs=1) as const,
            tc.tile_pool(name="big", bufs=1) as big,
            tc.tile_pool(name="stat", bufs=1) as stat,
            tc.tile_pool(name="stp", bufs=2) as stp,
            tc.tile_pool(name="outp", bufs=2) as outp,
        ):
            with tc.tile_pool(name="p1", bufs=1, space="PSUM") as p1:
                # ---------------- phase 0: loads -------------------------------
                x_sb = big.tile([C, HW], F32R, tag="x")
                ctx_sb = big.tile([C, HW], F32R, tag="ctx")
                for j in range(8):
                    nc.sync.dma_start(out=x_sb[:, ts(j, 512)], in_=x_d[:, ts(j, 512)])
                    nc.sync.dma_start(out=ctx_sb[:, ts(j, 512)], in_=ctx_d[:, ts(j, 512)])
                wq4_sb = const.tile([C, C], F32R, tag="wq4")
                nc.sync.dma_start(out=wq4_sb, in_=wq4_d[:])
                wk4_sb = const.tile([C, NH, C], F32R, tag="wk4")
                nc.sync.dma_start(out=wk4_sb, in_=wk4_d[:])
                wvt_sb = const.tile([C, HD], F32R, tag="wvt")
                nc.sync.dma_start(out=wvt_sb, in_=wvt_d[:])
                wot_sb = const.tile([HD, C], F32R, tag="wot")
                nc.sync.dma_start(out=wot_sb, in_=wot_d[:])
                gsel_sb = const.tile([C, C], F32, tag="gsel")
                nc.sync.dma_start(out=gsel_sb, in_=gsel_d[:])

                vecs = {}
                for name, d in (("gq", gq_d), ("bq", bq_d), ("gc", gc_d),
                                ("bc", bc_d), ("bo", bo_d)):
                    t = const.tile([C, 1], F32, tag=name)
                    nc.sync.dma_start(out=t, in_=d[:])
                    vecs[name] = t
                al_sb = const.tile([C, 1], F32, tag="al")
                nc.sync.dma_start(
                    out=al_sb,
                    in_=bass.AP(tensor=al_d[:].tensor, offset=0, ap=[[0, C], [1, 1]]),
                )
                rw_sb = const.tile([C, 1], F32, tag="rw")
                nc.sync.dma_start(
                    out=rw_sb,
                    in_=bass.AP(tensor=rw_d[:].tensor, offset=0, ap=[[0, C], [1, 1]]),
                )
                eps_sb = const.tile([C, 1], F32, tag="eps")
                nc.vector.memset(eps_sb, EPS)
                ones_sb = const.tile([33, C], F32, tag="ones")
                nc.vector.memset(ones_sb[32:33, :], 1.0)

                # ---------------- phase 1: groupnorm stats → folded weights ----
                def gn_fold(src_sb, gamma, beta, tagp):
                    # per-channel mean / E[x^2] via bn_stats, group-combined via
                    # the gsel matmul (gsel[i,j] = 0.25 * same_group(i,j)).
                    stats = stat.tile([C, 8, 6], F32, tag=f"bns{tagp}")
                    srcv = src_sb.bitcast(F32).rearrange("c (n f) -> c n f", f=512)
                    for i in range(8):
                        nc.vector.bn_stats(out=stats[:, i, :], in_=srcv[:, i, :])
                    mv = stat.tile([C, 2], F32, tag=f"mv{tagp}")
                    nc.vector.bn_aggr(out=mv, in_=stats)
                    ms = stat.tile([C, 2], F32, tag=f"ms{tagp}")
                    nc.vector.tensor_copy(out=ms[:, 0:1], in_=mv[:, 0:1])
                    nc.vector.tensor_mul(out=ms[:, 1:2], in0=mv[:, 0:1], in1=mv[:, 0:1])
                    nc.vector.tensor_add(out=ms[:, 1:2], in0=ms[:, 1:2], in1=mv[:, 1:2])
                    gp = p1.tile([C, 2], F32, tag="gp")
                    nc.tensor.matmul(gp, lhsT=gsel_sb, rhs=ms, start=True, stop=True)
                    gm = stat.tile([C, 2], F32, tag=f"gm{tagp}")
                    nc.vector.tensor_copy(out=gm, in_=gp)
                    varg = stat.tile([C, 1], F32, tag=f"vg{tagp}")
                    nc.vector.tensor_mul(out=varg, in0=gm[:, 0:1], in1=gm[:, 0:1])
                    nc.vector.tensor_sub(out=varg, in0=gm[:, 1:2], in1=varg)
                    # rstd = exp(-0.5 * ln(var + eps)); keeps everything in the
                    # natural_log_exp table set shared with the softmax exp.
                    lnv = stat.tile([C, 1], F32, tag=f"ln{tagp}")
                    nc.scalar.activation(out=lnv, in_=varg, func=AF.Ln, bias=eps_sb, scale=1.0)
                    rstd = stat.tile([C, 1], F32, tag=f"rs{tagp}")
                    nc.scalar.activation(out=rstd, in_=lnv, func=AF.Exp, bias=0.0, scale=-0.5)
                    s1 = stat.tile([C, 1], F32, tag=f"s1{tagp}")
                    nc.vector.tensor_mul(out=s1, in0=rstd, in1=gamma)
                    s0 = stat.tile([C, 1], F32, tag=f"s0{tagp}")
                    nc.vector.tensor_mul(out=s0, in0=gm[:, 0:1], in1=s1)
                    nc.vector.tensor_sub(out=s0, in0=beta, in1=s0)
                    return s1, s0

                s1q, s0q = gn_fold(x_sb, vecs["gq"], vecs["bq"], "q")
                s1k, s0k = gn_fold(ctx_sb, vecs["gc"], vecs["bc"], "k")

                # projection biases (with unfolded weights), then fold s1 into W
                qbp = p1.tile([C, 512], F32, tag="p1b")
                nc.tensor.matmul(qbp[:, 0:1], lhsT=wq4_sb.bitcast(F32), rhs=s0q, start=True, stop=True)
                qb = stat.tile([C, 1], F32, tag="qb")
                nc.vector.tensor_copy(out=qb, in_=qbp[:, 0:1])
                kbp = p1.tile([C, 512], F32, tag="p1b")
                for g in range(NH):
                    nc.tensor.matmul(kbp[:, 0:1], lhsT=wk4_sb[:, g, :].bitcast(F32), rhs=s0k,
                                     start=(g == 0), stop=(g == NH - 1))
                kb = stat.tile([C, 1], F32, tag="kb")
                nc.vector.tensor_copy(out=kb, in_=kbp[:, 0:1])
                nc.vector.tensor_scalar_mul(out=wq4_sb, in0=wq4_sb.bitcast(F32), scalar1=s1q)
                nc.vector.tensor_scalar_mul(
                    out=wk4_sb.rearrange("c g i -> c (g i)"),
                    in0=wk4_sb.bitcast(F32).rearrange("c g i -> c (g i)"), scalar1=s1k)

                # fold alpha into wot / bout; resw gates residual + bias
                nc.vector.tensor_scalar_mul(out=wot_sb, in0=wot_sb.bitcast(F32), scalar1=al_sb[0:HD])
                bout_sr = stat.tile([C, 1], F32, tag="bosr")
                nc.vector.tensor_mul(out=bout_sr, in0=vecs["bo"], in1=al_sb)
                nc.vector.tensor_mul(out=bout_sr, in0=bout_sr, in1=rw_sb)

                # ---------------- phase 2: projections -------------------------
                q_rep = big.tile([C, HW], BF16, tag="qrep")
                for j in range(8):
                    qp = p1.tile([C, 512], F32, tag="p1b")
                    nc.tensor.matmul(qp, lhsT=wq4_sb, rhs=x_sb[:, ts(j, 512)],
                                     start=True, stop=True)
                    nc.scalar.activation(out=q_rep[:, ts(j, 512)], in_=qp,
                                         func=AF.Identity, bias=qb, scale=1.0)

                # k distributed: e-tile eo lives on partitions 32*(eo%4).. ,
                # free slot eo//4.  ctx viewed as (c, bo, g, ei).
                kdist = big.tile([C, 8, 128], BF16, tag="kdist")
                ctx4 = ctx_sb.rearrange("c (bo g ei) -> c bo g ei", g=NH, ei=128)
                kdp = p1.tile([C, 8, 128], F32, tag="p1a")
                for half in range(2):
                    for g in range(NH):
                        nc.tensor.matmul(
                            kdp[:, half * 4:(half + 1) * 4, :],
                            lhsT=wk4_sb[:, g, :],
                            rhs=ctx4[:, half * 4:(half + 1) * 4, g, :],
                            start=(g == 0), stop=(g == NH - 1))
                nc.scalar.activation(out=kdist, in_=kdp, func=AF.Identity,
                                     bias=kb, scale=1.0)

                # v^T (+ ones row for the softmax denominator)
                vt = big.tile([C, NE, HD + 1], F32R, tag="vt")
                ctxe = ctx_sb.rearrange("c (eo ei) -> c eo ei", ei=128)
                for half in range(2):
                    vp = p1.tile([C, 512], F32, tag="p1b")
                    for i in range(16):
                        eo = half * 16 + i
                        nc.tensor.matmul(vp[:, ts(i, HD)], lhsT=ctxe[:, eo, :],
                                         rhs=wvt_sb, start=True, stop=True)
                    nc.vector.tensor_copy(
                        out=vt[:, half * 16:(half + 1) * 16, 0:HD],
                        in_=vp.rearrange("c (i v) -> c i v", v=HD))
                ones1 = const.tile([C, 1], F32, tag="one1")
                nc.vector.memset(ones1, 1.0)
                nc.vector.tensor_copy(
                    out=vt[:, :, HD:HD + 1],
                    in_=ones1[:, None, :].to_broadcast([C, NE, 1]))

                # x := x * resw (residual gate; all reads of raw x are done)
                nc.vector.tensor_scalar_mul(out=x_sb, in0=x_sb.bitcast(F32), scalar1=rw_sb)

            with (
                tc.tile_pool(name="spA", bufs=1, space="PSUM") as spA,
                tc.tile_pool(name="spB", bufs=1, space="PSUM") as spB,
                tc.tile_pool(name="avp", bufs=1, space="PSUM") as avp,
                tc.tile_pool(name="tlp", bufs=1, space="PSUM") as tlp,
            ):
                # ---------------- phase 3: attention ---------------------------
                # The PE is in-order, so everything that waits on another
                # engine is software-pipelined behind PE work:
                #  - AV(g) is emitted two exp-groups behind the score fills
                #    (exp(g) ran while fills g+1, g+2 executed);
                #  - the previous chunk's tail matmuls (1/L broadcast, out
                #    projection) are emitted in the middle of this chunk's
                #    group loop, long after their DVE inputs completed.
                # Otherwise the PE stalls >3.4us and HAM halves its clock.
                bounds = []
                eo = 0
                for size, which in EXP_GROUPS:
                    bounds.append((eo, size, which))
                    eo += size

                pend = {}  # previous chunk's tail state

                def tail_bc(s):
                    # 1/L broadcast: rbc = ones^T @ rinv (full fp32)
                    s["rbc"] = tlp.tile([C, D], F32, tag="tl", name="rbc")
                    nc.tensor.matmul(s["rbc"], lhsT=ones_sb[32:33, :],
                                     rhs=s["rinv"][HD:HD + 1, :],
                                     start=True, stop=True)
                    s["onrm"] = outp.tile([HD, D], F32R, tag="on", name="onrm")
                    nc.vector.tensor_mul(out=s["onrm"], in0=s["out_sb"][0:HD, :],
                                         in1=s["rbc"][0:HD, :])

                def tail_proj(s):
                    dcp = s["dc"]
                    yp = tlp.tile([C, D], F32, tag="tl")
                    nc.tensor.matmul(yp, lhsT=wot_sb, rhs=s["onrm"],
                                     start=True, stop=True)
                    y_sb = outp.tile([C, D], F32, tag="y")
                    nc.vector.tensor_scalar_add(out=y_sb, in0=yp, scalar1=bout_sr)
                    nc.vector.tensor_add(out=y_sb, in0=y_sb,
                                         in1=x_sb.bitcast(F32)[:, ts(dcp, D)])
                    nc.sync.dma_start(out=y_d[:, ts(dcp, D)], in_=y_sb)

                for dc in range(ND):
                    st = stp.tile([C, NE, D], F32R, tag="st")
                    av = avp.tile([C, D], F32, tag="av")

                    def av_group(gi):
                        e0, sz, _ = bounds[gi]
                        for e in range(e0, e0 + sz):
                            nc.tensor.matmul(av[0:HD + 1, :], lhsT=vt[:, e, :],
                                             rhs=st[:, e, :],
                                             start=(e == 0), stop=(e == NE - 1))

                    for gi, (eo, size, which) in enumerate(bounds):
                        pool = spA if which == "A" else spB
                        sp = pool.tile([C, size * D], F32, tag=which)
                        for i in range(size):
                            e = eo + i
                            g = e % 4
                            nc.tensor.matmul(
                                sp[:, ts(i, D)],
                                lhsT=kdist[32 * g:32 * (g + 1), e // 4, :],
                                rhs=q_rep[32 * g:32 * (g + 1), ts(dc, D)],
                                start=True, stop=True,
                                tile_position=(32 * g, 0))
                        nc.scalar.activation(
                            out=st[:, eo:eo + size, :],
                            in_=sp.rearrange("c (a b) -> c a b", b=D),
                            func=AF.Exp, bias=0.0, scale=SCALE)
                        if gi == 2 and pend:
                            tail_bc(pend)
                        if gi == 5 and pend:
                            tail_proj(pend)
                        if gi >= 2:
                            av_group(gi - 2)
                    av_group(len(bounds) - 2)
                    av_group(len(bounds) - 1)
                    out_sb = outp.tile([HD + 1, D], F32, tag="o")
                    nc.vector.tensor_copy(out=out_sb, in_=av[0:HD + 1, :])
                    rinv = outp.tile([HD + 1, D], F32, tag="ri")
                    nc.vector.reciprocal(out=rinv[HD:HD + 1, :],
                                         in_=out_sb[HD:HD + 1, :])
                    pend = {"dc": dc, "out_sb": out_sb, "rinv": rinv}
                # flush the last chunk's tail
                tail_bc(pend)
                tail_proj(pend)

    nc.compile()
    return nc


_CACHE = {}


def _get_module():
    if "nc" not in _CACHE:
        _CACHE["nc"] = _build_module()
    return _CACHE["nc"]


def _make_in_maps(inputs):
    f = lambda a: np.ascontiguousarray(np.asarray(a, dtype=np.float32))
    x = f(inputs["x"]).reshape(B, C, HW)
    ctx = f(inputs["context"]).reshape(B, C, HW)
    Wq, Wk, Wv, Wout = f(inputs["Wq"]), f(inputs["Wk"]), f(inputs["Wv"]), f(inputs["Wout"])
    gq, bq, gc, bc = f(inputs["gq"]), f(inputs["bq"]), f(inputs["gctx"]), f(inputs["bctx"])
    bo, al = f(inputs["bout"]), f(inputs["alpha"]).reshape(1, 1)

    gi = np.arange(C) // (C // NG)
    gsel = (gi[:, None] == gi[None, :]).astype(np.float32) / (C // NG)

    in_maps = []
    for core in range(8):
        b, h = core // NH, core % NH
        sl = slice(h * HD, (h + 1) * HD)
        wqT = np.ascontiguousarray(Wq[sl, :].T)           # (C, HD)
        wq4 = np.ascontiguousarray(np.tile(wqT, (1, NH)))  # (C, C) replicated
        wkT = np.ascontiguousarray(Wk[sl, :].T)
        wk4 = np.zeros((C, NH, C), np.float32)
        for g in range(NH):
            wk4[:, g, 32 * g:32 * (g + 1)] = wkT
        in_maps.append({
            "x": x[b].copy(),
            "ctx": ctx[b].copy(),
            "wq4": wq4,
            "wk4": wk4,
            "wvt": np.ascontiguousarray(Wv[sl, :].T),
            "wot": np.ascontiguousarray(Wout[:, sl].T),
            "gsel": gsel.copy(),
            "gq": gq.reshape(C, 1).copy(),
            "bq": bq.reshape(C, 1).copy(),
            "gc": gc.reshape(C, 1).copy(),
            "bc": bc.reshape(C, 1).copy(),
            "bo": bo.reshape(C, 1).copy(),
            "al": al.copy(),
            "rw": np.array([[1.0 if h == 0 else 0.0]], np.float32),
        })
    return in_maps


def run_full(inputs, trace=False, **kw):
    nc = _get_module()
    in_maps = _make_in_maps(inputs)
    res = run_bass_kernel_spmd(nc, in_maps, core_ids=list(range(8)),
                               trace=trace, **kw)
    out = np.zeros((B, C, HW), np.float32)
    for core in range(8):
        out[core // NH] += res.results[core]["y"]
    return out.reshape(B, C, H, W), res


def kernel(**inputs) -> np.ndarray:
    out, _ = run_full(inputs, trace=False)
    return out



# revision 8
# speedup vs baseline: 1.2060x; 1.2060x over previous
"""Trainium2 Bass kernel for a cross-attention block (B=2, C=128, H=W=64, 4 heads).

Sharding: one (batch, head) pair per NeuronCore (2*4 = 8 cores).  Each core:
  - group-norms x[b] / context[b] (stats only; the affine normalization is
    folded into the projection weights),
  - computes its head's q, k, v projections,
  - runs softmax(q^T k / sqrt(hd)) @ v^T with the score matrix streamed
    through PSUM (never materialized in HBM),
  - applies its head's slice of the output projection.
The host sums the 4 per-head partial outputs of each batch (the residual x
and bias are added on exactly one core per batch via the `resw` input, so the
sum is a pure unshard).

Softmax exp is split across TWO engines so neither is the bottleneck:
  - A-groups (4 e-tiles, 4 PSUM banks) -> ScalarE ACT exp.  Scores arrive
    pre-scaled by 2^23*log2(e)/sqrt(hd) (folded into the q projection), so
    ACT computes exp(ln2/2^23 * T + ln2/2) = 2^(t + 0.5).
  - B-groups (2 e-tiles, 2 PSUM banks) -> a custom VectorE (DVE) op that
    evaluates 2^(t+0.5) in ONE 8-stage pass using the magic-number
    float->int trick: u = T + 1.5*2^46 captures round(t)*2^23 exactly;
    F = T - nf is the fractional part *2^23; a quadratic in F builds the
    IEEE-754 mantissa and the int32 *output conversion* acts as the final
    bitcast (max rel err 1.7e-3, diluted ~100x by softmax averaging).
  The constant 2^0.5 factor cancels in softmax (numerator and the ones-row
  denominator are scaled identically).

Layout notes:
  - Scores are computed transposed (e on partitions, d free) so softmax
    normalization uses a ones-row appended to v^T (column sums fall out of
    the same matmul as attn@v) and no transposes are needed anywhere.
  - S^T matmuls have contraction dim 32 (head dim); 4 consecutive e-tiles
    are packed into the 4 PE row groups (tile_position) so they run
    concurrently: k is produced in a "distributed" layout and q replicated
    on all 4 partition groups, both directly from the projection matmuls.
  - 1/L uses reciprocal_approx_fast (custom DVE op, ~5x faster).
  - x/ctx are loaded as half/quarter tiles so bn_stats and the v projection
    overlap the input DMA; the residual gate+bias fold runs on GpSimd.
"""

import numpy as np

import concourse.bass as bass
import concourse.bacc as bacc
import concourse.tile as tile
import concourse.mybir as mybir
from concourse.bass import ts
from concourse.bass_utils import run_bass_kernel_spmd

import concourse.dve_ops as dve_ops_mod
from concourse.dve_spec import Spec, Src0, C0, C1, C2, C3, _spill_c3_to_src1
from concourse.dve_ops import DveOp

F32 = mybir.dt.float32
F32R = mybir.dt.float32r
I32 = mybir.dt.int32
I16 = mybir.dt.int16
BF16 = mybir.dt.bfloat16
AF = mybir.ActivationFunctionType
OP = mybir.AluOpType

B, C, H, W = 2, 128, 64, 64
HW = H * W            # 4096
NH = 4                # heads
HD = C // NH          # 32
NG = 32               # groupnorm groups
EPS = 1e-5
NE = HW // 128        # 32 e-tiles of 128
D = 512               # d-chunk (query positions per chunk)
ND = HW // D          # 8 chunks
SCALE = float(1.0 / np.sqrt(HD))
LN2 = float(np.log(2.0))
# scores arrive as T = t * 2^7 with t in log2 units: fold into q weights.
# 2^7 (not 2^23) because st is bf16: the custom DVE op emits int16 whose bits
# are the TOP half of the fp32 pattern, i.e. a bf16 bitcast.
BETA = float((2.0 ** 7) * SCALE / LN2)

# custom DVE exp2: quadratic mantissa fit p(f) ~ 2^(f+0.5), f in [-0.5, 0.5)
K0, K1, K2 = 1.414839858227856, 0.9948160429319775, 0.3371845243305162
MAGIC = float(1.5 * 2 ** 30)
C1V = float((126.0 + K0) * 2 ** 7)
C2V = float(K2 / 2 ** 7)

# exp groups per chunk: A-groups (4 e-tiles) go to ScalarE, B-groups (2) to
# the DVE custom op.  spA = 4 banks, spB = 2 banks (+1 av +1 tail = 8).
EXP_GROUPS = [(4, "A"), (2, "B"), (4, "A"), (2, "B"), (4, "A"), (2, "B"),
              (4, "A"), (2, "B"), (4, "A"), (2, "B"), (2, "B")]


def _exp2_ref(in0, in1, s0, s1, imm2):
    T = in0.astype(np.float32)
    u = np.float32(T + np.float32(s0))
    nf = np.float32(u - np.float32(s0))
    F = np.float32(T - nf)
    k1v = np.asarray(in1, np.float32).reshape(-1, 1)
    return np.float32(
        np.float32(np.float32(np.float32(F * np.float32(imm2)) + k1v) * F) + nf
    ) + np.float32(s1)


_u = Src0 + C0
_nf = _u - C0
_F = Src0 - _nf
EXP2F_ANT = DveOp(
    "EXP2F_ANT",
    Spec(body=_spill_c3_to_src1((_F * C2 + C3) * _F + _nf + C1), reference=_exp2_ref),
    subdim=False,
    uops_sha={"v3": "03226ada4f820bbd", "v4": "082478e9f10bfe3d"},
)
if EXP2F_ANT.name not in dve_ops_mod._SUB_OPCODE_FOR_NAME:
    dve_ops_mod.OPS.append(EXP2F_ANT)
    dve_ops_mod._SUB_OPCODE_FOR_NAME[EXP2F_ANT.name] = (
        dve_ops_mod._CUSTOM_DVE_ROW_BASE + len(dve_ops_mod.OPS) - 1
    )
    dve_ops_mod.CUSTOM_DVE_SPECS[EXP2F_ANT.name] = EXP2F_ANT.spec


def _r(ap):
    return ap.bitcast(F32R)


def _build_module():
    nc = bacc.Bacc("TRN2", target_bir_lowering=False)

    x_d = nc.dram_tensor("x", (C, HW), F32R, kind="ExternalInput")
    ctx_d = nc.dram_tensor("ctx", (C, HW), F32R, kind="ExternalInput")
    wq4_d = nc.dram_tensor("wq4", (C, C), F32R, kind="ExternalInput")
    wk4_d = nc.dram_tensor("wk4", (C, NH, C), F32R, kind="ExternalInput")
    wvt_d = nc.dram_tensor("wvt", (C, HD), F32R, kind="ExternalInput")
    wot_d = nc.dram_tensor("wot", (HD, C), F32R, kind="ExternalInput")
    gsel_d = nc.dram_tensor("gsel", (C, C), F32, kind="ExternalInput")
    gq_d = nc.dram_tensor("gq", (C, 1), F32, kind="ExternalInput")
    bq_d = nc.dram_tensor("bq", (C, 1), F32, kind="ExternalInput")
    gc_d = nc.dram_tensor("gc", (C, 1), F32, kind="ExternalInput")
    bc_d = nc.dram_tensor("bc", (C, 1), F32, kind="ExternalInput")
    bo_d = nc.dram_tensor("bo", (C, 1), F32, kind="ExternalInput")
    al_d = nc.dram_tensor("al", (1, 1), F32, kind="ExternalInput")
    rw_d = nc.dram_tensor("rw", (1, 1), F32, kind="ExternalInput")
    y_d = nc.dram_tensor("y", (C, HW), F32, kind="ExternalOutput")

    with tile.TileContext(nc) as tc:
        with (
            tc.tile_pool(name="const", bufs=1) as const,
            tc.tile_pool(name="big", bufs=1) as big,
            tc.tile_pool(name="stat", bufs=1) as stat,
            tc.tile_pool(name="stp", bufs=2) as stp,
            tc.tile_pool(name="outp", bufs=2) as outp,
        ):
            with tc.tile_pool(name="p1", bufs=1, space="PSUM") as p1:
                # ---------------- phase 0: loads -------------------------------
                # small weights first so projections can start ASAP
                wvt_sb = const.tile([C, HD], F32R, tag="wvt")
                nc.sync.dma_start(out=wvt_sb, in_=wvt_d[:])
                wq4_sb = const.tile([C, C], F32R, tag="wq4")
                nc.sync.dma_start(out=wq4_sb, in_=wq4_d[:])
                wk4_sb = const.tile([C, NH, C], F32R, tag="wk4")
                nc.sync.dma_start(out=wk4_sb, in_=wk4_d[:])
                wot_sb = const.tile([HD, C], F32R, tag="wot")
                nc.sync.dma_start(out=wot_sb, in_=wot_d[:])
                gsel_sb = const.tile([C, C], F32, tag="gsel")
                nc.sync.dma_start(out=gsel_sb, in_=gsel_d[:])

                vecs = {}
                for name, d in (("gq", gq_d), ("bq", bq_d), ("gc", gc_d),
                                ("bc", bc_d), ("bo", bo_d)):
                    t = const.tile([C, 1], F32, tag=name)
                    nc.sync.dma_start(out=t, in_=d[:])
                    vecs[name] = t
                al_sb = const.tile([C, 1], F32, tag="al")
                nc.sync.dma_start(
                    out=al_sb,
                    in_=bass.AP(tensor=al_d[:].tensor, offset=0, ap=[[0, C], [1, 1]]),
                )
                rw_sb = const.tile([C, 1], F32, tag="rw")
                nc.sync.dma_start(
                    out=rw_sb,
                    in_=bass.AP(tensor=rw_d[:].tensor, offset=0, ap=[[0, C], [1, 1]]),
                )
                eps_sb = const.tile([C, 1], F32, tag="eps")
                nc.vector.memset(eps_sb, EPS)
                hb_sb = const.tile([C, 1], F32, tag="hb")
                nc.vector.memset(hb_sb, 0.5 * LN2)
                k1_sb = const.tile([C, 1], F32, tag="k1c")
                nc.vector.memset(k1_sb, K1)
                ones_sb = const.tile([33, C], F32, tag="ones")
                nc.vector.memset(ones_sb[32:33, :], 1.0)
                ones1 = const.tile([C, 1], F32, tag="one1")
                nc.vector.memset(ones1, 1.0)

                # ctx as 2 half tiles (k-proj needs 2048-wide views),
                # x as 4 quarter tiles (finer DMA/stats overlap).
                ctx_h = []
                for h in range(2):
                    t = big.tile([C, HW // 2], F32R, tag=f"ctx{h}")
                    nc.sync.dma_start(out=t, in_=ctx_d[:, ts(h, HW // 2)])
                    ctx_h.append(t)
                x_q = []
                for qq in range(4):
                    t = big.tile([C, HW // 4], F32R, tag=f"x{qq}")
                    nc.sync.dma_start(out=t, in_=x_d[:, ts(qq, HW // 4)])
                    x_q.append(t)

                # ---------------- phase 1a: v projection (needs only raw ctx) --
                # v^T (+ ones row for the softmax denominator); per half so it
                # overlaps the other half's DMA and warms the PE early.
                vt = big.tile([C, NE, HD + 1], BF16, tag="vt")
                for half in range(2):
                    ctxe = ctx_h[half].rearrange("c (eo ei) -> c eo ei", ei=128)
                    vp = p1.tile([C, 512], F32, tag="p1b")
                    for i in range(16):
                        nc.tensor.matmul(vp[:, ts(i, HD)], lhsT=ctxe[:, i, :],
                                         rhs=wvt_sb, start=True, stop=True)
                    nc.vector.tensor_copy(
                        out=vt[:, half * 16:(half + 1) * 16, 0:HD],
                        in_=vp.rearrange("c (i v) -> c i v", v=HD))
                nc.vector.tensor_copy(
                    out=vt[:, :, HD:HD + 1],
                    in_=ones1[:, None, :].to_broadcast([C, NE, 1]))

                # ---------------- phase 1b: groupnorm stats → folded weights ---
                def gn_fold(parts, gamma, beta, tagp):
                    # per-channel mean / E[x^2] via bn_stats, group-combined via
                    # the gsel matmul (gsel[i,j] = 0.25 * same_group(i,j)).
                    # bn_stats is HW-capped at FD=512: sub-chunk each part
                    nsub = sum(p.shape[-1] // 512 for p in parts)
                    stats = stat.tile([C, nsub, 6], F32, tag=f"bns{tagp}")
                    i = 0
                    for part in parts:
                        pv = part.bitcast(F32).rearrange("c (n f) -> c n f", f=512)
                        for j in range(part.shape[-1] // 512):
                            nc.vector.bn_stats(out=stats[:, i, :], in_=pv[:, j, :])
                            i += 1
                    mv = stat.tile([C, 2], F32, tag=f"mv{tagp}")
                    nc.vector.bn_aggr(out=mv, in_=stats)
                    ms = stat.tile([C, 2], F32, tag=f"ms{tagp}")
                    nc.vector.tensor_copy(out=ms[:, 0:1], in_=mv[:, 0:1])
                    nc.vector.tensor_mul(out=ms[:, 1:2], in0=mv[:, 0:1], in1=mv[:, 0:1])
                    nc.vector.tensor_add(out=ms[:, 1:2], in0=ms[:, 1:2], in1=mv[:, 1:2])
                    gp = p1.tile([C, 2], F32, tag="gp")
                    nc.tensor.matmul(gp, lhsT=gsel_sb, rhs=ms, start=True, stop=True)
                    gm = stat.tile([C, 2], F32, tag=f"gm{tagp}")
                    nc.vector.tensor_copy(out=gm, in_=gp)
                    varg = stat.tile([C, 1], F32, tag=f"vg{tagp}")
                    nc.vector.tensor_mul(out=varg, in0=gm[:, 0:1], in1=gm[:, 0:1])
                    nc.vector.tensor_sub(out=varg, in0=gm[:, 1:2], in1=varg)
                    # rstd = exp(-0.5 * ln(var + eps)); keeps everything in the
                    # natural_log_exp table set shared with the softmax exp.
                    lnv = stat.tile([C, 1], F32, tag=f"ln{tagp}")
                    nc.scalar.activation(out=lnv, in_=varg, func=AF.Ln, bias=eps_sb, scale=1.0)
                    rstd = stat.tile([C, 1], F32, tag=f"rs{tagp}")
                    nc.scalar.activation(out=rstd, in_=lnv, func=AF.Exp, bias=0.0, scale=-0.5)
                    s1 = stat.tile([C, 1], F32, tag=f"s1{tagp}")
                    nc.vector.tensor_mul(out=s1, in0=rstd, in1=gamma)
                    s0 = stat.tile([C, 1], F32, tag=f"s0{tagp}")
                    nc.vector.tensor_mul(out=s0, in0=gm[:, 0:1], in1=s1)
                    nc.vector.tensor_sub(out=s0, in0=beta, in1=s0)
                    return s1, s0

                s1k, s0k = gn_fold(ctx_h, vecs["gc"], vecs["bc"], "k")
                s1q, s0q = gn_fold(x_q, vecs["gq"], vecs["bq"], "q")
                # fold the 2^23*log2(e)/sqrt(hd) score scale into the q side
                nc.vector.tensor_scalar_mul(out=s1q, in0=s1q, scalar1=BETA)
                nc.vector.tensor_scalar_mul(out=s0q, in0=s0q, scalar1=BETA)

                # projection biases (with unfolded weights), then fold s1 into W
                kbp = p1.tile([C, 512], F32, tag="p1b")
                for g in range(NH):
                    nc.tensor.matmul(kbp[:, 0:1], lhsT=wk4_sb[:, g, :].bitcast(F32), rhs=s0k,
                                     start=(g == 0), stop=(g == NH - 1))
                kb = stat.tile([C, 1], F32, tag="kb")
                nc.vector.tensor_copy(out=kb, in_=kbp[:, 0:1])
                nc.vector.tensor_scalar_mul(
                    out=wk4_sb.rearrange("c g i -> c (g i)"),
                    in0=wk4_sb.bitcast(F32).rearrange("c g i -> c (g i)"), scalar1=s1k)

                qbp = p1.tile([C, 512], F32, tag="p1b")
                nc.tensor.matmul(qbp[:, 0:1], lhsT=wq4_sb.bitcast(F32), rhs=s0q, start=True, stop=True)
                qb = stat.tile([C, 1], F32, tag="qb")
                nc.vector.tensor_copy(out=qb, in_=qbp[:, 0:1])
                nc.vector.tensor_scalar_mul(out=wq4_sb, in0=wq4_sb.bitcast(F32), scalar1=s1q)

                # fold alpha into wot / bout; resw gates residual + bias
                nc.vector.tensor_scalar_mul(out=wot_sb, in0=wot_sb.bitcast(F32), scalar1=al_sb[0:HD])
                bout_sr = stat.tile([C, 1], F32, tag="bosr")
                nc.vector.tensor_mul(out=bout_sr, in0=vecs["bo"], in1=al_sb)
                nc.vector.tensor_mul(out=bout_sr, in0=bout_sr, in1=rw_sb)

                # ---------------- phase 2: k/q projections ---------------------
                # k distributed: e-tile eo lives on partitions 32*(eo%4).. ,
                # free slot eo//4.  ctx half viewed as (c, bo, g, ei).
                kdist = big.tile([C, 8, 128], BF16, tag="kdist")
                kdp = p1.tile([C, 8, 128], F32, tag="p1a")
                for half in range(2):
                    ctx4 = ctx_h[half].rearrange("c (bo g ei) -> c bo g ei", g=NH, ei=128)
                    for g in range(NH):
                        nc.tensor.matmul(
                            kdp[:, half * 4:(half + 1) * 4, :],
                            lhsT=wk4_sb[:, g, :],
                            rhs=ctx4[:, :, g, :],
                            start=(g == 0), stop=(g == NH - 1))
                nc.scalar.activation(out=kdist, in_=kdp, func=AF.Identity,
                                     bias=kb, scale=1.0)

                q_rep = big.tile([C, HW], BF16, tag="qrep")
                for j in range(8):
                    qp = p1.tile([C, 512], F32, tag="p1b")
                    nc.tensor.matmul(qp, lhsT=wq4_sb, rhs=x_q[j // 2][:, ts(j % 2, 512)],
                                     start=True, stop=True)
                    nc.scalar.activation(out=q_rep[:, ts(j, 512)], in_=qp,
                                         func=AF.Identity, bias=qb, scale=1.0)

                # x' := x*resw + bout (residual gate + bias fold) on GpSimd —
                # keeps the DVE free for the softmax exp stream.
                import os as _os
                _gate_eng = nc.vector if _os.environ.get("NO_GPSIMD_GATE") == "1" else nc.gpsimd
                for qq in range(4):
                    _gate_eng.tensor_scalar(
                        out=x_q[qq], in0=x_q[qq].bitcast(F32),
                        scalar1=rw_sb, scalar2=bout_sr,
                        op0=OP.mult, op1=OP.add)

            with (
                tc.tile_pool(name="spA", bufs=1, space="PSUM") as spA,
                tc.tile_pool(name="spB", bufs=1, space="PSUM") as spB,
                tc.tile_pool(name="avp", bufs=1, space="PSUM") as avp,
                tc.tile_pool(name="tlp", bufs=1, space="PSUM") as tlp,
            ):
                # ---------------- phase 3: attention ---------------------------
                # Per chunk: PE fills score groups; ScalarE exps A-groups,
                # DVE exps B-groups (custom op); AV lags 2 groups behind.
                # The previous chunk's tail (copy/recip/normalize/project) is
                # threaded through both the DVE stream and the PE stream at
                # fixed points so nothing stalls.
                bounds = []
                eo = 0
                for size, which in EXP_GROUPS:
                    bounds.append((eo, size, which))
                    eo += size

                pend = {}  # previous chunk's tail state

                def tail_copy(s):
                    # av PSUM -> SBUF (PE can't read PSUM; frees av for reuse)
                    s["out_sb"] = outp.tile([HD + 1, D], F32, tag="o", name="out_sb")
                    nc.vector.tensor_copy(out=s["out_sb"], in_=s["av"][0:HD + 1, :])

                def tail_recip(s):
                    # custom-DVE ops crash on base_partition != 0: reciprocal
                    # the whole 33-row tile (rows 0..31 are junk, only the L
                    # row 32 is ever read by the rbc broadcast matmul).
                    s["rinv"] = outp.tile([HD + 1, D], F32, tag="ri", name="rinv")
                    nc.vector.reciprocal_approx_fast(
                        out=s["rinv"], in_=s["out_sb"])

                def tail_bc(s):
                    # 1/L broadcast: rbc = ones^T @ rinv (full fp32)
                    s["rbc"] = tlp.tile([C, D], F32, tag="tl", name="rbc")
                    nc.tensor.matmul(s["rbc"], lhsT=ones_sb[32:33, :],
                                     rhs=s["rinv"][HD:HD + 1, :],
                                     start=True, stop=True)

                def tail_onrm(s):
                    s["onrm"] = outp.tile([HD, D], F32R, tag="on", name="onrm")
                    nc.vector.tensor_mul(out=s["onrm"], in0=s["out_sb"][0:HD, :],
                                         in1=s["rbc"][0:HD, :])

                def tail_proj(s):
                    s["yp"] = tlp.tile([C, D], F32, tag="tl", name="yp")
                    nc.tensor.matmul(s["yp"], lhsT=wot_sb, rhs=s["onrm"],
                                     start=True, stop=True)

                def tail_out(s):
                    dcp = s["dc"]
                    y_sb = outp.tile([C, D], F32, tag="y")
                    nc.vector.tensor_add(out=y_sb, in0=s["yp"],
                                         in1=x_q[dcp // 2].bitcast(F32)[:, ts(dcp % 2, D)])
                    nc.sync.dma_start(out=y_d[:, ts(dcp, D)], in_=y_sb)

                for dc in range(ND):
                    st = stp.tile([C, NE, D], BF16, tag="st")
                    av = avp.tile([C, D], F32, tag="av")

                    def av_group(gi):
                        e0, sz, _ = bounds[gi]
                        for e in range(e0, e0 + sz):
                            nc.tensor.matmul(av[0:HD + 1, :], lhsT=vt[:, e, :],
                                             rhs=st[:, e, :],
                                             start=(e == 0), stop=(e == NE - 1))

                    nB = 0
                    for gi, (eo, size, which) in enumerate(bounds):
                        pool = spA if which == "A" else spB
                        sp = pool.tile([C, size * D], F32, tag=which)
                        for i in range(size):
                            e = eo + i
                            g = e % 4
                            nc.tensor.matmul(
                                sp[:, ts(i, D)],
                                lhsT=kdist[32 * g:32 * (g + 1), e // 4, :],
                                rhs=q_rep[32 * g:32 * (g + 1), ts(dc, D)],
                                start=True, stop=True,
                                tile_position=(32 * g, 0))
                        if which == "A":
                            nc.scalar.activation(
                                out=st[:, eo:eo + size, :],
                                in_=sp.rearrange("c (a b) -> c a b", b=D),
                                func=AF.Exp, bias=hb_sb, scale=LN2 / 2 ** 7)
                        else:
                            if nB == 0 and pend:
                                tail_copy(pend)
                            import os as _os
                            if _os.environ.get("NO_DVE_EXP") == "1":
                                nc.scalar.activation(
                                    out=st[:, eo:eo + size, :],
                                    in_=sp.rearrange("c (a b) -> c a b", b=D),
                                    func=AF.Exp, bias=hb_sb, scale=LN2 / 2 ** 7)
                            else:
                                nc.vector._custom_dve(
                                    EXP2F_ANT,
                                    out=st[:, eo:eo + size, :]
                                        .rearrange("c a b -> c (a b)").bitcast(I16),
                                    in0=sp,
                                    in1=k1_sb,
                                    s0=MAGIC, s1=C1V, imm2=C2V)
                            nB += 1
                            if pend:
                                if nB == 2:
                                    tail_recip(pend)
                                elif nB == 3:
                                    tail_onrm(pend)
                                elif nB == 4:
                                    tail_out(pend)
                        # PE-side tail matmuls at fixed fill points
                        if gi == 4 and pend:
                            tail_bc(pend)
                        if gi == 6 and pend:
                            tail_proj(pend)
                        if gi >= 2:
                            av_group(gi - 2)
                    av_group(len(bounds) - 2)
                    av_group(len(bounds) - 1)
                    pend = {"dc": dc, "av": av}
                # flush the last chunk's tail
                tail_copy(pend)
                tail_recip(pend)
                tail_bc(pend)
                tail_onrm(pend)
                tail_proj(pend)
                tail_out(pend)

    nc.compile()
    return nc


_CACHE = {}


def _get_module():
    if "nc" not in _CACHE:
        _CACHE["nc"] = _build_module()
    return _CACHE["nc"]


def _make_in_maps(inputs):
    f = lambda a: np.ascontiguousarray(np.asarray(a, dtype=np.float32))
    x = f(inputs["x"]).reshape(B, C, HW)
    ctx = f(inputs["context"]).reshape(B, C, HW)
    Wq, Wk, Wv, Wout = f(inputs["Wq"]), f(inputs["Wk"]), f(inputs["Wv"]), f(inputs["Wout"])
    gq, bq, gc, bc = f(inputs["gq"]), f(inputs["bq"]), f(inputs["gctx"]), f(inputs["bctx"])
    bo, al = f(inputs["bout"]), f(inputs["alpha"]).reshape(1, 1)

    gi = np.arange(C) // (C // NG)
    gsel = (gi[:, None] == gi[None, :]).astype(np.float32) / (C // NG)

    in_maps = []
    for core in range(8):
        b, h = core // NH, core % NH
        sl = slice(h * HD, (h + 1) * HD)
        wqT = np.ascontiguousarray(Wq[sl, :].T)           # (C, HD)
        wq4 = np.ascontiguousarray(np.tile(wqT, (1, NH)))  # (C, C) replicated
        wkT = np.ascontiguousarray(Wk[sl, :].T)
        wk4 = np.zeros((C, NH, C), np.float32)
        for g in range(NH):
            wk4[:, g, 32 * g:32 * (g + 1)] = wkT
        in_maps.append({
            "x": x[b].copy(),
            "ctx": ctx[b].copy(),
            "wq4": wq4,
            "wk4": wk4,
            "wvt": np.ascontiguousarray(Wv[sl, :].T),
            "wot": np.ascontiguousarray(Wout[:, sl].T),
            "gsel": gsel.copy(),
            "gq": gq.reshape(C, 1).copy(),
            "bq": bq.reshape(C, 1).copy(),
            "gc": gc.reshape(C, 1).copy(),
            "bc": bc.reshape(C, 1).copy(),
            "bo": bo.reshape(C, 1).copy(),
            "al": al.copy(),
            "rw": np.array([[1.0 if h == 0 else 0.0]], np.float32),
        })
    return in_maps


def run_full(inputs, trace=False, **kw):
    nc = _get_module()
    in_maps = _make_in_maps(inputs)
    res = run_bass_kernel_spmd(nc, in_maps, core_ids=list(range(8)),
                               trace=trace, **kw)
    out = np.zeros((B, C, HW), np.float32)
    for core in range(8):
        out[core // NH] += res.results[core]["y"]
    return out.reshape(B, C, H, W), res


def kernel(**inputs) -> np.ndarray:
    out, _ = run_full(inputs, trace=False)
    return out
